# revision 9
# baseline (speedup 1.0000x reference)
"""CSWin cross-window attention (idx==1) + LePE depthwise conv, Trainium2 Bass kernel.

Problem: temp (16,3,128,64,64) fp32 -> out (16,4096,128) fp32.
Windows: H_sp=64, W_sp=8 -> nW=8 vertical stripes, L=512 tokens/window,
4 heads x 32 dims. 128 windows total, data-parallel 16 windows/core on 8 cores.

Per-core device pipeline, per window (all matmuls bf16, fp32 PSUM accum):
  QK^T   : row-tiled 4-head matmuls (K=32) -> ST [keys,queries] in PSUM
  exp    : ScalarE activation, scale=1/sqrt(32) fused, fp32 PSUM -> bf16 SBUF
  AV     : col-tiled 4-head matmuls (M=32), keys-contraction -> OT [C, L]
  denom  : same streams against an all-ones [128,32] stationary -> sum_k exp,
           broadcast across each head's 32 partitions by the matmul itself
  conv   : depthwise 3x3 as 9 diagonal-stationary matmuls over shifted views
           (+1 bias matmul against a ones rhs)
  combine: out = OT * recip(denom) + rpe on VectorE, DMA out as [C, L]
Host does the window split / bf16 cast on the way in and the [C,L]->(tok,C)
transpose on the way out.
"""
import sys

sys.path.insert(0, "/opt/trn_rl_repo")

import numpy as np
import ml_dtypes

import bass_rust
import concourse.bass as bass
import concourse.mybir as mybir
import concourse.tile as tile
from concourse.vector_clock import ScopedClock
from concourse.bass_utils import run_bass_kernel_spmd

BF16 = ml_dtypes.bfloat16

N_CORES = 8
B, C, H, W = 16, 128, 64, 64
NH, HD, WSP = 4, 32, 8
L = H * WSP          # 512 tokens per window
NWIN = 16            # windows per core
KC = 4               # key chunks of 128
SCALE = float(HD) ** -0.5

# taps: center first so its matmul (start=True) covers every output element
TAPS = [(0, 0), (-1, -1), (-1, 0), (-1, 1), (0, -1), (0, 1), (1, -1), (1, 0), (1, 1)]


class _TC(tile.TileContext):
    """Tail-drain patch: stock TileContext attaches every outstanding sem wait
    to a single Drain, but walrus caps non-EVSEM instructions at one wait."""

    def _drain_and_barrier(self, tick_clock, wait_clock):
        drain_inst = self.nc.sync.drain()
        wait_clock.add_sem_waits(
            drain_inst.ins, ScopedClock({None: tick_clock.global_clock})
        )
        si = drain_inst.ins.sync_info
        if si is not None and len(si.on_wait) > 1:
            waits = list(si.on_wait)
            drain_inst.ins.sync_info = bass_rust.SyncInfo(
                on_wait=[waits[0]], on_update=list(si.on_update)
            )
            for wv in waits[1:]:
                extra = self.nc.sync.drain()
                extra.ins.sync_info = bass_rust.SyncInfo(on_wait=[wv], on_update=[])
        self.nc.all_engine_barrier()
        assert self.sems is not None
        popped = self.nc._tile_sem_poison_stack.pop()
        assert popped is self._sem_poison
        self.nc.clear_and_free_semaphores(list(self.sems.allocated().values()))
        self.nc.all_engine_barrier()


def _split_dma_waits(nc):
    """walrus's DMA pseudo-instruction struct accepts a single sync wait;
    Tile can attach two. Hoist extras onto EventSemaphore no-ops just before."""
    for fn in nc.m.functions:
        for bb in fn.blocks:
            new = []
            for ins in bb.instructions:
                si = ins.sync_info
                if (
                    not isinstance(ins, mybir.InstEventSemaphore)
                    and si is not None
                    and len(si.on_wait) > 1
                ):
                    waits = list(si.on_wait)
                    for i in range(0, len(waits) - 1, 2):
                        ev = mybir.InstEventSemaphore(
                            name=nc.get_next_instruction_name(), ins=[], outs=[]
                        )
                        ev.engine = ins.engine
                        ev.sync_info = bass_rust.SyncInfo(
                            on_wait=waits[i : min(i + 2, len(waits) - 1)], on_update=[]
                        )
                        new.append(ev)
                    ins.sync_info = bass_rust.SyncInfo(
                        on_wait=[waits[-1]], on_update=list(si.on_update)
                    )
                new.append(ins)
            bb.instructions = new


def build_kernel():
    f32, bf = mybir.dt.float32, mybir.dt.bfloat16
    nc = bass.Bass("TRN2")

    qt_d = nc.dram_tensor("qt", [NWIN, C, L], bf, kind="ExternalInput")
    kt_d = nc.dram_tensor("kt", [NWIN, C, L], bf, kind="ExternalInput")
    vw_d = nc.dram_tensor("vw", [NWIN, C, L], bf, kind="ExternalInput")  # [keys, C]
    vc_d = nc.dram_tensor("vc", [NWIN, C, L], bf, kind="ExternalInput")  # [C, l]
    wdiag_d = nc.dram_tensor("wdiag", [C, 10 * C], bf, kind="ExternalInput")
    ones512_d = nc.dram_tensor("ones512", [C, L], bf, kind="ExternalInput")
    ones32_d = nc.dram_tensor("ones32", [C, HD], bf, kind="ExternalInput")
    out_d = nc.dram_tensor("out", [NWIN, C, L], f32, kind="ExternalOutput")

    from contextlib import ExitStack

    with _TC(nc) as tc, ExitStack() as ctx:
        cpool = ctx.enter_context(tc.tile_pool(name="consts", bufs=1))
        inp = ctx.enter_context(tc.tile_pool(name="inp", bufs=3))
        epool = ctx.enter_context(tc.tile_pool(name="epool", bufs=6))
        fpool = ctx.enter_context(tc.tile_pool(name="fpool", bufs=2))
        stp = ctx.enter_context(tc.tile_pool(name="stp", bufs=1, space="PSUM"))
        avp = ctx.enter_context(tc.tile_pool(name="avp", bufs=1, space="PSUM"))
        sdp = ctx.enter_context(tc.tile_pool(name="sdp", bufs=1, space="PSUM"))
        cvp = ctx.enter_context(tc.tile_pool(name="cvp", bufs=1, space="PSUM"))

        wdiag_sb = cpool.tile([C, 10 * C], bf)
        ones512_sb = cpool.tile([C, L], bf)
        ones32_sb = cpool.tile([C, HD], bf)
        nc.gpsimd.dma_start(wdiag_sb[:], wdiag_d[:])
        nc.gpsimd.dma_start(ones512_sb[:], ones512_d[:])
        nc.gpsimd.dma_start(ones32_sb[:], ones32_d[:])

        for w in range(NWIN):
            qt_sb = inp.tile([C, L], bf)
            kt_sb = inp.tile([C, L], bf)
            vw_sb = inp.tile([C, L], bf)
            vc_sb = inp.tile([C, L], bf)
            nc.gpsimd.dma_start(qt_sb[:], qt_d[w])
            nc.gpsimd.dma_start(kt_sb[:], kt_d[w])
            nc.gpsimd.dma_start(vw_sb[:], vw_d[w])
            nc.gpsimd.dma_start(vc_sb[:], vc_d[w])

            av_ps = avp.tile([C, L], f32)
            sden_ps = sdp.tile([C, L], f32)

            for kc in range(KC):
                st_t = stp.tile([C, NH, L], f32)  # 4 PSUM banks, one per head
                for h in range(NH):
                    nc.tensor.matmul(
                        st_t[:, h, :],
                        kt_sb[32 * h : 32 * h + 32, 128 * kc : 128 * kc + 128],
                        qt_sb[32 * h : 32 * h + 32, :],
                        start=True,
                        stop=True,
                        tile_position=(32 * h, 0),
                    )
                e_t = epool.tile([C, NH, L], bf)
                nc.scalar.activation(
                    e_t[:], st_t[:], mybir.ActivationFunctionType.Exp, scale=SCALE
                )
                for h in range(NH):
                    nc.tensor.matmul(
                        av_ps[32 * h : 32 * h + 32, :],
                        vw_sb[:, 128 * kc + 32 * h : 128 * kc + 32 * h + 32],
                        e_t[:, h, :],
                        start=(kc == 0),
                        stop=(kc == KC - 1),
                        tile_position=(0, 32 * h),
                    )
                for h in range(NH):
                    nc.tensor.matmul(
                        sden_ps[32 * h : 32 * h + 32, :],
                        ones32_sb[:],
                        e_t[:, h, :],
                        start=(kc == 0),
                        stop=(kc == KC - 1),
                        tile_position=(0, 32 * h),
                    )

            # depthwise 3x3 conv + bias, accumulated in PSUM
            conv_ps = cvp.tile([C, L], f32)
            cps = conv_ps[:].rearrange("p (y x) -> p y x", x=WSP)
            vcr = vc_sb[:].rearrange("p (y x) -> p y x", x=WSP)
            for ti, (dy, dx) in enumerate(TAPS):
                y0, y1 = max(0, -dy), H - max(0, dy)
                x0, x1 = max(0, -dx), WSP - max(0, dx)
                nc.tensor.matmul(
                    cps[:, y0:y1, x0:x1],
                    wdiag_sb[:, 128 * ti : 128 * ti + 128],
                    vcr[:, y0 + dy : y1 + dy, x0 + dx : x1 + dx],
                    start=(ti == 0),
                    stop=False,
                )
            nc.tensor.matmul(
                conv_ps[:],
                wdiag_sb[:, 128 * 9 : 128 * 10],
                ones512_sb[:],
                start=False,
                stop=True,
            )

            r_t = fpool.tile([C, L], f32)
            nc.vector.reciprocal(r_t[:], sden_ps[:])
            tmp_t = fpool.tile([C, L], f32)
            nc.vector.tensor_mul(tmp_t[:], av_ps[:], r_t[:])
            out_t = fpool.tile([C, L], f32)
            nc.vector.tensor_add(out_t[:], tmp_t[:], conv_ps[:])
            nc.gpsimd.dma_start(out_d[w], out_t[:])

    _split_dma_waits(nc)
    return nc


def prep_core_inputs(temp, conv_w, conv_b, core):
    """Host-side: slice core's 16 windows into device layouts, cast bf16."""
    qt = np.empty((NWIN, C, L), BF16)
    kt = np.empty((NWIN, C, L), BF16)
    vw = np.empty((NWIN, C, L), BF16)
    vc = np.empty((NWIN, C, L), BF16)
    for k in range(NWIN):
        g = core * NWIN + k
        b, wi = divmod(g, W // WSP)
        win = temp[b, :, :, :, WSP * wi : WSP * wi + WSP]  # (3, C, H, WSP)
        q2 = win[0].reshape(C, L)
        k2 = win[1].reshape(C, L)
        v2 = win[2].reshape(C, L)
        qt[k] = q2.astype(BF16)
        kt[k] = k2.astype(BF16)
        vc[k] = v2.astype(BF16)
        # vw[l, c] chunked: [keys 128, kc*128 + c]
        vw[k] = (
            v2.T.reshape(KC, 128, C).transpose(1, 0, 2).reshape(128, KC * C)
        ).astype(BF16)
    wdiag = np.zeros((C, 10 * C), BF16)
    idx = np.arange(C)
    ti = 0
    for dy, dx in TAPS:
        wdiag[idx, 128 * ti + idx] = conv_w[:, 0, dy + 1, dx + 1].astype(BF16)
        ti += 1
    wdiag[idx, 128 * 9 + idx] = conv_b.astype(BF16)
    return {
        "qt": qt,
        "kt": kt,
        "vw": vw,
        "vc": vc,
        "wdiag": wdiag,
        "ones512": np.ones((C, L), BF16),
        "ones32": np.ones((C, HD), BF16),
    }


_nc_cache = None


def kernel(temp, conv_w, conv_b, _want_profile=False, _trace_kwargs=None):
    global _nc_cache
    temp = np.asarray(temp, np.float32)
    conv_w = np.asarray(conv_w, np.float32)
    conv_b = np.asarray(conv_b, np.float32)

    if _nc_cache is None:
        _nc_cache = build_kernel()
    nc = _nc_cache

    in_maps = [prep_core_inputs(temp, conv_w, conv_b, r) for r in range(N_CORES)]
    kwargs = {}
    if _want_profile:
        kwargs = {"trace": True}
        if _trace_kwargs:
            kwargs.update(_trace_kwargs)
    res = run_bass_kernel_spmd(nc, in_maps, core_ids=list(range(N_CORES)), **kwargs)

    out = np.zeros((B, H, W, C), np.float32)
    for r in range(N_CORES):
        o = res.results[r]["out"]  # (16, C, L)
        for k in range(NWIN):
            g = r * NWIN + k
            b, wi = divmod(g, W // WSP)
            # [C, L] -> [H, WSP, C]
            out[b, :, WSP * wi : WSP * wi + WSP, :] = (
                o[k].reshape(C, H, WSP).transpose(1, 2, 0)
            )
    result = out.reshape(B, H * W, C)
    if _want_profile:
        return result, res
    return result
